# revision 32
# baseline (speedup 1.0000x reference)
"""Trainium2 Bass kernel for nn_Encoder_16578573763343 (dense transformer encoder).

Sharding: attention heads across the 8 cores (H == n_cores == 8), FFN
sequence-parallel on each core's 256 owned rows. Head outputs are combined
with 4 chunked bf16 ReduceScatters; core c owns original rows {512b + 64c + i}.
Global layernorm stats via a tiny AllGather.

Attention runs in fp8(e4m3) with DoubleRow matmuls (2 k-tiles/instr at
0.5 cyc/row): the q/k projections are folded into one matrix M = Wq @ Wk^T
on the host (scores = x M x^T), shipped as single-fp8; Wv ships single-fp8
with the systematic quantization bias x_bar @ (Wv - fp8(Wv)) folded into the
v bias on the host. Activations x, G=(xM)^T, P=exp(S/32), v are fp8.
The FFN stays bf16 (its error path is undamped). Scale bookkeeping: x,G,v
carry x16 scale, P carries x8, the residual z carries x16 (LN folds it out).

x^T is built with DMA-transposes of the gathered embedding rows plus a
host-pre-transposed positional table; v and G matmuls are interleaved with
the build so the PE fills early. W1 is prefetched before attention. The
LN AllGather hides under the U-B matmuls whose relu evictions are fused
tensor_scalar(add,max) ops (the LN scale is folded into the y eviction).

Self-contained: hardcodes all shapes from the problem spec.
"""

import os

import numpy as np
import ml_dtypes

S = 2048
D = 1024
H = 8
DFF = 3 * D
VOCAB = 32000
EPS = 1e-5
NCORES = 8
SL = S // NCORES  # 256 rows owned per core

P = 128
DT = D // P      # 8  d-tiles
ST = S // P      # 16 s-tiles
FT = DFF // P    # 24 f-tiles
SB = 512         # s-block for attention / matmul free dim
NB = S // SB     # 4 attention s-blocks
SLT = SL // P    # 2

SX = 16.0        # x, G, v, z scale
SW = 64.0        # M quant scale
SV = 32.0        # Wv quant scale
SP8 = 8.0        # P = exp scale


def _pos_encoding() -> np.ndarray:
    pos = np.arange(S, dtype=np.float32)[:, None]
    i = np.arange(D)
    angle = pos / np.power(10000.0, (2 * (i // 2)).astype(np.float32) / D)
    return np.where(i % 2 == 0, np.sin(angle), np.cos(angle)).astype(np.float32)


def _build():
    import concourse.mybir as mybir
    import concourse.tile as tile
    from concourse import bacc
    from concourse.bass import IndirectOffsetOnAxis
    from concourse.masks import make_identity

    # debug bisection stages: "x" < "qkv" < "attn" < "full"
    STAGE = os.environ.get("BASS_KERNEL_STAGE", "full")
    SVS = {"x": 0, "qkv": 1, "attn": 2, "full": 4}[STAGE]

    BF = mybir.dt.bfloat16
    F8 = mybir.dt.float8e4
    F32 = mybir.dt.float32
    I32 = mybir.dt.int32
    AF = mybir.ActivationFunctionType
    ALU = mybir.AluOpType
    DR = mybir.MatmulPerfMode.DoubleRow
    RG = [list(range(NCORES))]

    nc = bacc.Bacc(
        "TRN2",
        target_bir_lowering=False,
        debug=False,
        enable_asserts=False,
        num_devices=NCORES,
    )

    # ---- I/O (host prearranges layouts; see _prepare_in_maps) ----
    t_pm = nc.dram_tensor("tokens_pm", [P, ST], I32, kind="ExternalInput")
    t_sl = nc.dram_tensor("tokens_sl", [P, SLT], I32, kind="ExternalInput")
    emb = nc.dram_tensor("emb", [VOCAB, D], BF, kind="ExternalInput")   # x16
    posT = nc.dram_tensor("posT", [P, DT * S], BF, kind="ExternalInput")  # x16, transposed
    pos_slr = nc.dram_tensor("pos_slr", [P, 2 * D], BF, kind="ExternalInput")  # x16, row layout
    m8 = nc.dram_tensor("m8", [DT, P, DT * P], F8, kind="ExternalInput")
    v8 = nc.dram_tensor("v8", [2, P, DT * SB], F8, kind="ExternalInput")
    wexp = nc.dram_tensor("wexp", [P, ST], F32, kind="ExternalInput")
    bvs = nc.dram_tensor("bvs", [1, D], F32, kind="ExternalInput")      # x16, sum over heads
    w1 = nc.dram_tensor("w1", [FT, P, DT * P], BF, kind="ExternalInput")
    cs = nc.dram_tensor("cs", [P, FT], F32, kind="ExternalInput")
    b1 = nc.dram_tensor("b1", [P, FT], F32, kind="ExternalInput")
    w2 = nc.dram_tensor("w2", [FT, P, D], BF, kind="ExternalInput")
    b2 = nc.dram_tensor("b2", [1, D], F32, kind="ExternalInput")
    out = nc.dram_tensor("out", [SL, D], F32, kind="ExternalOutput")

    with tile.TileContext(nc) as tc:
        with tc.tile_pool(name="const", bufs=1) as const, \
             tc.tile_pool(name="persist", bufs=1) as persist, \
             tc.tile_pool(name="dram", bufs=1, space="DRAM") as dram:

            # ---- constants ----
            ident_f = const.tile([P, P], F32, name="ident_f")
            make_identity(nc, ident_f[:])
            ident_bf = const.tile([P, P], BF, name="ident_bf")
            nc.vector.tensor_copy(out=ident_bf[:], in_=ident_f[:])
            ones_blk_f = const.tile([P, 64], F32, name="ones_blk_f")
            nc.vector.memset(ones_blk_f[:], 1.0)

            tok_pm = const.tile([P, ST], I32, name="tok_pm")
            nc.sync.dma_start(tok_pm[:], t_pm[:, :])
            tok_sl = const.tile([P, SLT], I32, name="tok_sl")
            wexp_t = const.tile([P, ST], F32, name="wexp_t")
            b1_t = const.tile([P, FT], F32, name="b1_t")
            cs_t = const.tile([P, FT], F32, name="cs_t")
            bvs_bc = const.tile([P, D], F32, name="bvs_bc")
            b2_bc = const.tile([P, D], F32, name="b2_bc")
            inv32 = const.tile([P, 1], F32, name="inv32")
            nc.vector.memset(inv32[:], 1.0 / SV)

            # ---- persistent tensors ----
            v = persist.tile([P, ST, D + 64], F8, name="v")
            nc.vector.memset(v[:, :, D : D + 64], 1.0)
            xbrow = persist.tile([P, 2, D], BF, name="xbrow")   # x16 + bvs, row layout
            abs_bc = persist.tile([P, 3], F32, name="abs_bc")

            # ---- internal DRAM ----
            o_rs_b = [dram.tile([2 * SB, D], BF, name=f"o_rs_{b}", tag=f"o_rs_{b}") for b in range(2)]
            mha_b = [dram.tile([P, D], BF, name=f"mha_{b}", tag=f"mha_{b}") for b in range(2)]
            st_in = dram.tile([1, 8], F32, name="st_in")
            st_out = dram.tile([8, 8], F32, name="st_out", addr_space="Shared")

            # ====== Phase 1+2 interleaved: x^T build + v + G per block ======
            with tc.tile_pool(name="w1p", bufs=1) as w1p:
              with tc.tile_pool(name="xTp", bufs=1) as xTp:
                xT = xTp.tile([P, DT, S], F8, name="xT")
                G8 = xTp.tile([P, DT, S], F8, name="G8")
                w1ts = []
                if SVS >= 3:
                    for fm in range(FT):
                        w1ts.append(w1p.tile([P, DT, P], BF, name=f"w1t{fm}"))

                pP_cm = tc.tile_pool(name="pP", bufs=2)
                pP = pP_cm.__enter__()
                ps_s_cm = tc.tile_pool(name="ps_s", bufs=3, space="PSUM")
                ps_s = ps_s_cm.__enter__()

                def scores_tile(b, t, Pt):
                    ps = ps_s.tile([P, SB], F32, name="ps_s_t", tag="ps_s_t")
                    for kd in range(DT // 2):
                        nc.tensor.matmul(
                            ps[:],
                            lhsT=xT[:, 2 * kd : 2 * kd + 2, t * P : (t + 1) * P],
                            rhs=G8[:, 2 * kd : 2 * kd + 2, b * SB : (b + 1) * SB],
                            start=(kd == 0),
                            stop=(kd == DT // 2 - 1),
                            perf_mode=DR,
                        )
                    nc.scalar.activation(
                        Pt[:, t, :], ps[:], AF.Exp,
                        bias=wexp_t[:, t : t + 1],
                        scale=1.0 / (SX * SX * 32.0),
                    )

                Pt0 = pP.tile([P, ST, SB], F8, name="Pt", tag="Pt") if SVS >= 2 else None

                with tc.tile_pool(name="ph1", bufs=3) as ph1, \
                     tc.tile_pool(name="wqk", bufs=1) as wp, \
                     tc.tile_pool(name="psqk", bufs=2, space="PSUM") as psq, \
                     tc.tile_pool(name="psv", bufs=3, space="PSUM") as psv:
                    HS = S // 4
                    def load_posT_half(h):
                        pt = ph1.tile([P, DT, HS], BF, name="posT_h", tag="posT_h", bufs=2)
                        nc.sync.dma_start(
                            pt[:], posT.ap().rearrange("p (k s) -> p k s", s=S)[:, :, h * HS : (h + 1) * HS]
                        )
                        return pt
                    # critical loads only: wv (v needs it at iter 0) and posT chunk 0
                    wvts = []
                    for n2 in range(2):
                        wvt = wp.tile([P, DT, SB], F8, name=f"wv{n2}")
                        nc.sync.dma_start(wvt[:], v8.ap()[n2].rearrange("p (k e) -> p k e", e=SB))
                        wvts.append(wvt)
                    posT_h = load_posT_half(0)
                    wms = [wp.tile([P, DT, P], F8, name=f"wm{m}") for m in range(DT)]

                    def deferred_loads(t):
                        # stagger weight/const loads behind the first transposes
                        if t == 1:
                            for m in range(DT // 2):
                                nc.sync.dma_start(wms[m][:], m8.ap()[m].rearrange("p (k e) -> p k e", e=P))
                        elif t == 2:
                            for m in range(DT // 2, DT):
                                nc.sync.dma_start(wms[m][:], m8.ap()[m].rearrange("p (k e) -> p k e", e=P))
                        elif t == 3:
                            nc.sync.dma_start(tok_sl[:], t_sl[:, :])
                            nc.sync.dma_start(wexp_t[:], wexp[:, :])
                            nc.sync.dma_start(b1_t[:], b1[:, :])
                            nc.sync.dma_start(cs_t[:], cs[:, :])
                        elif t == 4:
                            bvs_row = ph1.tile([1, D], F32, name="bvs_row", tag="bvs_row", bufs=1)
                            nc.sync.dma_start(bvs_row[:], bvs[:, :])
                            nc.gpsimd.partition_broadcast(bvs_bc[:], bvs_row[:])
                            b2_t = ph1.tile([1, D], F32, name="b2_t", tag="b2_t", bufs=1)
                            nc.sync.dma_start(b2_t[:], b2[:, :])
                            nc.gpsimd.partition_broadcast(b2_bc[:], b2_t[:])

                    def gather_emb(t):
                        embt = ph1.tile([P, D], BF, name="embt", tag="embt")
                        nc.gpsimd.indirect_dma_start(
                            out=embt[:],
                            out_offset=None,
                            in_=emb.ap(),
                            in_offset=IndirectOffsetOnAxis(ap=tok_pm[:, t : t + 1], axis=0),
                        )
                        return embt

                    # software-pipelined: gathers lead the transpose/add by 3
                    embts = [gather_emb(0), gather_emb(1), gather_emb(2)]
                    for t in range(ST):
                        deferred_loads(t)
                        if t + 3 < ST:
                            embts.append(gather_emb(t + 3))
                        if t % 4 == 2 and t < ST - 4:
                            posT_nh = load_posT_half(t // 4 + 1)
                        if t % 4 == 0 and t > 0:
                            posT_h = posT_nh
                        embT = ph1.tile([P, DT, P], BF, name="embT", tag="embT")
                        nc.sync.dma_start(embT[:], embts[t][:], transpose=True)
                        th = t % 4
                        # x8^T slice = fp8(embT + posT)
                        nc.vector.tensor_add(
                            out=xT[:, :, t * P : (t + 1) * P],
                            in0=embT[:],
                            in1=posT_h[:, :, th * P : (th + 1) * P],
                        )
                        if SVS >= 1:
                            # v row-tile t (single-fp8 Wv; bias+corr via STT in1)
                            for n2 in range(2):
                                ps = psv.tile([P, SB], F32, name="psv_t", tag="psv_t")
                                for kd in range(DT // 2):
                                    nc.tensor.matmul(
                                        ps[:],
                                        lhsT=xT[:, 2 * kd : 2 * kd + 2, t * P : (t + 1) * P],
                                        rhs=wvts[n2][:, 2 * kd : 2 * kd + 2, :],
                                        start=(kd == 0),
                                        stop=(kd == DT // 2 - 1),
                                        perf_mode=DR,
                                    )
                                if n2 == 0:
                                    nc.scalar.activation(
                                        v[:, t, 0:SB], ps[:], AF.Identity,
                                        bias=0.0, scale=1.0 / SV,
                                    )
                                else:
                                    nc.vector.tensor_scalar_mul(
                                        v[:, t, SB : 2 * SB], ps[:], 1.0 / SV
                                    )
                            if SVS >= 2 and t >= 4:
                                # early block-0 scores as xT tiles land
                                scores_tile(0, t - 4, Pt0)
                            if t % 4 == 3:
                                # G columns for block b = t//4 (single-fp8 M)
                                b = t // 4
                                for m in range(DT):
                                    ps = psq.tile([P, SB], F32, name="psqk_t", tag="psqk_t")
                                    for kd in range(DT // 2):
                                        nc.tensor.matmul(
                                            ps[:],
                                            lhsT=wms[m][:, 2 * kd : 2 * kd + 2, :],
                                            rhs=xT[:, 2 * kd : 2 * kd + 2, b * SB : (b + 1) * SB],
                                            start=(kd == 0),
                                            stop=(kd == DT // 2 - 1),
                                            perf_mode=DR,
                                        )
                                    nc.scalar.activation(
                                        G8[:, m, b * SB : (b + 1) * SB],
                                        ps[:],
                                        AF.Identity,
                                        bias=0.0,
                                        scale=1.0 / SW,
                                    )

                    # x rows for the residual (this core's 256 owned rows), row layout
                    possr = ph1.tile([P, 2, D], BF, name="possr", tag="possr", bufs=1)
                    nc.sync.dma_start(possr[:], pos_slr[:, :])
                    for j in range(SLT):
                        embs = ph1.tile([P, D], BF, name="embs", tag="embt")
                        nc.gpsimd.indirect_dma_start(
                            out=embs[:],
                            out_offset=None,
                            in_=emb.ap(),
                            in_offset=IndirectOffsetOnAxis(ap=tok_sl[:, j : j + 1], axis=0),
                        )
                        xsum = ph1.tile([P, D], F32, name="xsum", tag="xsum")
                        nc.vector.tensor_add(out=xsum[:], in0=embs[:], in1=possr[:, j, :])
                        nc.vector.tensor_add(
                            out=xbrow[:, j, :], in0=xsum[:], in1=bvs_bc[:]
                        )

                # ---- prefetch W1 (tiles pre-created; transfers run under attention) ----
                if SVS >= 3:
                    for fm in range(FT):
                        nc.sync.dma_start(w1ts[fm][:], w1.ap()[fm].rearrange("p (k e) -> p k e", e=P))

                # =========== Phase 3: attention + chunked RS ===========
                if SVS >= 2:
                    # S^T[t, s] = x G per s-block; P^T = 8*exp(S^T/32); o = P^T.T @ v
                    with tc.tile_pool(name="ps_o", bufs=3, space="PSUM") as ps_o, \
                         tc.tile_pool(name="ps_r", bufs=2, space="PSUM") as ps_r, \
                         tc.tile_pool(name="oev", bufs=4) as oev:
                        for b in range(NB):
                            if b == 0:
                                Pt = Pt0
                                for t in range(ST - 4, ST):
                                    scores_tile(0, t, Pt)
                            else:
                                Pt = pP.tile([P, ST, SB], F8, name="Pt", tag="Pt")
                                for t in range(ST):
                                    scores_tile(b, t, Pt)
                            for sm in range(SB // P):
                                po0 = ps_o.tile([P, SB], F32, name="po0", tag="po")
                                po1 = ps_o.tile([P, SB], F32, name="po1", tag="po")
                                pr = ps_r.tile([P, 64], F32, name="pr", tag="pr")
                                for t in range(ST // 2):
                                    lh = Pt[:, 2 * t : 2 * t + 2, sm * P : (sm + 1) * P]
                                    st0 = t == 0
                                    st1 = t == ST // 2 - 1
                                    nc.tensor.matmul(po0[:], lhsT=lh, rhs=v[:, 2 * t : 2 * t + 2, 0:SB],
                                                     start=st0, stop=st1, perf_mode=DR)
                                    nc.tensor.matmul(po1[:], lhsT=lh, rhs=v[:, 2 * t : 2 * t + 2, SB : 2 * SB],
                                                     start=st0, stop=st1, perf_mode=DR)
                                    nc.tensor.matmul(pr[:], lhsT=lh, rhs=v[:, 2 * t : 2 * t + 2, D : D + 64],
                                                     start=st0, stop=st1, perf_mode=DR)
                                ot = oev.tile([P, 2, SB], BF, name="ot", tag="oevt")
                                recip = oev.tile([P, 1], F32, name="recip", tag="recip")
                                nc.vector.reciprocal(recip[:], pr[:, 0:1])
                                nc.scalar.mul(ot[:, 0, :], po0[:], recip[:, 0:1])
                                nc.vector.tensor_scalar_mul(ot[:, 1, :], po1[:], recip[:, 0:1])
                                nc.sync.dma_start(o_rs_b[b // 2][(b % 2) * SB + sm * P : (b % 2) * SB + (sm + 1) * P, :], ot[:])
                            if SVS >= 3 and b % 2 == 1:
                                # chunked RS: rank c receives original rows {1024j + 128c + i}
                                nc.gpsimd.collective_compute(
                                    "ReduceScatter",
                                    ALU.add,
                                    replica_groups=RG,
                                    ins=[o_rs_b[b // 2][:]],
                                    outs=[mha_b[b // 2][:]],
                                )

                ps_s_cm.__exit__(None, None, None)
                pP_cm.__exit__(None, None, None)

                if SVS < 3:
                    # debug stages: write something derived from the last-built tensor
                    with tc.tile_pool(name="dbg", bufs=2) as dbg:
                        if SVS == 0:
                            for j in range(SLT):
                                f0 = dbg.tile([P, D], F32, name="f0", tag="f0")
                                nc.vector.tensor_copy(out=f0[:], in_=xTsl[:, :, j * P : (j + 1) * P])
                                nc.sync.dma_start(out.ap()[j * P : (j + 1) * P, :], f0[:])
                        elif SVS == 1:
                            f0 = dbg.tile([P, D], F32, name="f0", tag="f0")
                            nc.vector.tensor_copy(out=f0[:], in_=G8[:, :, 0:P])
                            nc.sync.dma_start(out.ap()[0:P, :], f0[:])
                            f1 = dbg.tile([P, D], F32, name="f1", tag="f0")
                            nc.vector.tensor_copy(out=f1[:], in_=v[:, 0, 0:D])
                            nc.sync.dma_start(out.ap()[P : 2 * P, :], f1[:])
                        elif SVS == 2:
                            for j in range(SLT):
                                f0 = dbg.tile([P, D], BF, name="f0", tag="f0")
                                nc.sync.dma_start(f0[:], o_rs_b[0][j * P : (j + 1) * P, :])
                                f1 = dbg.tile([P, D], F32, name="f1", tag="f1")
                                nc.vector.tensor_copy(out=f1[:], in_=f0[:])
                                nc.sync.dma_start(out.ap()[j * P : (j + 1) * P, :], f1[:])
              # xT, G8, Pt freed here (w1p still open)

              # ===== Phase 4+5: residual, U = z@W1 (hides RS/AG), LN, FFN =====
              if SVS >= 3:
                with tc.tile_pool(name="w2p", bufs=1) as w2p:
                  hT = w2p.tile([P, FT, SL], BF, name="hT")
                  with tc.tile_pool(name="upool", bufs=1) as upool, \
                       tc.tile_pool(name="mr", bufs=1) as mr, \
                       tc.tile_pool(name="ph4", bufs=1) as ph4, \
                       tc.tile_pool(name="ps_mr", bufs=2, space="PSUM") as ps_mr, \
                       tc.tile_pool(name="psA", bufs=2, space="PSUM") as psA, \
                       tc.tile_pool(name="ps4", bufs=1, space="PSUM") as ps4:
                    U_sb = [upool.tile([P, FT, P], BF, name=f"U_sb{j}") for j in range(2)]
                    zbf = upool.tile([P, DT, SL], BF, name="zbf")
                    zrow = upool.tile([P, 2, D], BF, name="zrow")
                    sqs = upool.tile([P, D], BF, name="sqs")

                    def chunk_residual(j):
                        mch = mr.tile([P, D], F32, name="mch", tag="mch")
                        nc.gpsimd.dma_start(mch[:], mha_b[j][:])
                        nc.vector.tensor_add(out=zrow[:, j, :], in0=mch[:], in1=xbrow[:, j, :])

                    def chunk_zbf(j):
                        for d in range(DT):
                            psm = ps_mr.tile([P, P], BF, name="psm", tag="psm")
                            nc.tensor.transpose(psm[:], zrow[:, j, d * P : (d + 1) * P], ident_bf[:])
                            nc.vector.tensor_copy(
                                out=zbf[:, d, P * j : P * (j + 1)], in_=psm[:]
                            )

                    def u_part(j):
                        for fm in range(FT):
                            ps = psA.tile([P, P], F32, name="psA_t", tag="psA_t")
                            for kd in range(DT):
                                nc.tensor.matmul(
                                    ps[:],
                                    lhsT=w1ts[fm][:, kd, :],
                                    rhs=zbf[:, kd, P * j : P * (j + 1)],
                                    start=(kd == 0),
                                    stop=(kd == DT - 1),
                                )
                            nc.vector.tensor_copy(out=U_sb[j][:, fm, :], in_=ps[:])

                    red = ph4.tile([P, 4], F32, name="red", tag="red")

                    def chunk_stats(j):
                        nc.vector.tensor_reduce(
                            red[:, 2 * j : 2 * j + 1],
                            zrow[:, j, :],
                            axis=mybir.AxisListType.X, op=ALU.add,
                        )
                        nc.scalar.activation(
                            sqs[:], zrow[:, j, :],
                            AF.Square, accum_out=red[:, 2 * j + 1 : 2 * j + 2],
                        )

                    chunk_residual(0)
                    chunk_stats(0)
                    chunk_zbf(0)
                    # U part A: chunk-0 columns — runs while RS(1) lands
                    u_part(0)
                    chunk_residual(1)
                    chunk_stats(1)
                    pst = ps4.tile([64, 4], F32, name="pst", tag="pst")
                    nc.tensor.matmul(pst[:], lhsT=ones_blk_f[:], rhs=red[:], start=True, stop=True)
                    st_sb = ph4.tile([1, 8], F32, name="st_sb", tag="st_sb")
                    nc.vector.memset(st_sb[:], 0.0)
                    pst_sb = ph4.tile([1, 4], F32, name="pst_sb", tag="pst_sb")
                    nc.vector.tensor_copy(out=pst_sb[:], in_=pst[0:1, 0:4])
                    nc.vector.tensor_add(
                        out=st_sb[0:1, 0:2], in0=pst_sb[:, 0:2], in1=pst_sb[:, 2:4]
                    )
                    nc.gpsimd.dma_start(st_in[:], st_sb[:])
                    nc.gpsimd.collective_compute(
                        "AllGather",
                        ALU.bypass,
                        replica_groups=RG,
                        ins=[st_in[:]],
                        outs=[st_out[:]],
                    )
                    # U part B: chunk-1 transposes + columns — AG hides here
                    chunk_zbf(1)
                    u_part(1)
                    # prefetch W2 while the AllGather is in flight
                    w2ts = []
                    for kf in range(FT):
                        w2t = w2p.tile([P, D], BF, name=f"w2t{kf}")
                        nc.sync.dma_start(w2t[:], w2.ap()[kf])
                        w2ts.append(w2t)

                    # AG readback + LN coefficients
                    stg = ph4.tile([1, 64], F32, name="stg")
                    nc.scalar.dma_start(stg[:], st_out[:])
                    # T1 = 16*sum(z), T2 = 256*sum(z^2) across all cores
                    t1 = ph4.tile([1, 1], F32, name="t1")
                    nc.vector.tensor_reduce(t1[:], stg[:, 0:64:8], axis=mybir.AxisListType.X, op=ALU.add)
                    t2 = ph4.tile([1, 1], F32, name="t2")
                    nc.vector.tensor_reduce(t2[:], stg[:, 1:64:8], axis=mybir.AxisListType.X, op=ALU.add)
                    invSD = 1.0 / float(S * D)
                    abs_t = ph4.tile([1, 3], F32, name="abs_t")     # [a16, -16mean, 16sd]
                    mean_t = ph4.tile([1, 1], F32, name="mean_t")   # 16*mean
                    nc.vector.tensor_scalar_mul(mean_t[:], t1[:], invSD)
                    nc.vector.tensor_scalar_mul(abs_t[:, 1:2], t1[:], -invSD)
                    msq = ph4.tile([1, 1], F32, name="msq")
                    nc.vector.tensor_mul(out=msq[:], in0=mean_t[:], in1=mean_t[:])
                    var = ph4.tile([1, 1], F32, name="var")         # 256*var
                    nc.vector.scalar_tensor_tensor(
                        out=var[:], in0=t2[:], scalar=invSD, in1=msq[:],
                        op0=ALU.mult, op1=ALU.subtract,
                    )
                    eps_t = ph4.tile([1, 1], F32, name="eps_t")
                    nc.vector.memset(eps_t[:], EPS * SX * SX)
                    nc.scalar.activation(abs_t[:, 2:3], var[:], AF.Sqrt, bias=eps_t[:], scale=1.0)
                    nc.vector.reciprocal(abs_t[:, 0:1], abs_t[:, 2:3])
                    nc.gpsimd.partition_broadcast(abs_bc[:], abs_t[:])
                    # biasf2 = -16mean*cs + 16sd*b1  (for h' = relu(U16 + biasf2))
                    b1s = ph4.tile([P, FT], F32, name="b1s")
                    nc.vector.tensor_scalar_mul(b1s[:], b1_t[:], abs_bc[:, 2:3])
                    biasf2 = ph4.tile([P, FT], F32, name="biasf2")
                    nc.vector.scalar_tensor_tensor(
                        out=biasf2[:],
                        in0=cs_t[:],
                        scalar=abs_bc[:, 1:2],
                        in1=b1s[:],
                        op0=ALU.mult,
                        op1=ALU.add,
                    )
                    # h'^T = relu(U16 + biasf2): chunk 0 on Pool, chunk 1 on DVE
                    for fm in range(FT):
                        nc.gpsimd.tensor_scalar(
                            hT[:, fm, 0:P], U_sb[0][:, fm, :],
                            biasf2[:, fm : fm + 1], 0.0,
                            op0=ALU.add, op1=ALU.max,
                        )
                        nc.vector.tensor_scalar(
                            hT[:, fm, P : 2 * P], U_sb[1][:, fm, :],
                            biasf2[:, fm : fm + 1], 0.0,
                            op0=ALU.add, op1=ALU.max,
                        )
                  # phase-4 PSUM pools freed; FFN2 gets its own banks
                  with tc.tile_pool(name="ps_y", bufs=1, space="PSUM") as ps_y, \
                       tc.tile_pool(name="yev", bufs=2) as yev:
                    # FFN out: y = a16*(h' @ W2) + b2
                    pys = {}
                    for sm in range(SLT):
                        for dn in range(2):
                            pys[(sm, dn)] = ps_y.tile(
                                [P, SB], F32, name=f"py_{sm}_{dn}", tag=f"py_{sm}_{dn}"
                            )
                    for kf in range(FT):
                        for sm in range(SLT):
                            for dn in range(2):
                                nc.tensor.matmul(
                                    pys[(sm, dn)][:],
                                    lhsT=hT[:, kf, sm * P : (sm + 1) * P],
                                    rhs=w2ts[kf][:, dn * SB : (dn + 1) * SB],
                                    start=(kf == 0),
                                    stop=(kf == FT - 1),
                                )
                    for sm in range(SLT):
                        y = yev.tile([P, 2, SB], F32, name="y", tag="y")
                        for dn in range(2):
                            nc.vector.scalar_tensor_tensor(
                                out=y[:, dn, :],
                                in0=pys[(sm, dn)][:],
                                scalar=abs_bc[:, 0:1],
                                in1=b2_bc[:, dn * SB : (dn + 1) * SB],
                                op0=ALU.mult,
                                op1=ALU.add,
                            )
                        nc.sync.dma_start(out.ap()[sm * P : (sm + 1) * P, :], y[:])

    nc.compile()
    return nc


_CACHE = {}


def _get_module():
    if "nc" not in _CACHE:
        _CACHE["nc"] = _build()
    return _CACHE["nc"]


def _owned_rows(c: int) -> np.ndarray:
    """Original row indices owned by core c, in local order l = 128j + i."""
    l = np.arange(SL)
    return 1024 * (l // P) + P * c + (l % P)


def _prepare_in_maps(inputs):
    bf = ml_dtypes.bfloat16
    f8 = ml_dtypes.float8_e4m3
    tokens = np.asarray(inputs["tokens"], dtype=np.int32)
    emb_f = np.asarray(inputs["emb"], dtype=np.float32)
    emb16 = np.ascontiguousarray(emb_f * SX).astype(bf)
    pe = _pos_encoding()

    W1 = np.asarray(inputs["W1"], np.float32)
    W2 = np.asarray(inputs["W2"], np.float32)

    def _dT_layout(A):
        # A [S', D] -> [P, DT*S']: out[p, d*S'+s] = A[s, d*128+p]
        Sp = A.shape[0]
        return np.ascontiguousarray(
            A.reshape(Sp, DT, P).transpose(2, 1, 0).reshape(P, DT * Sp)
        )

    # tokens arranged [p, n] = tokens[n*128 + p] so gather call n covers s-tile n
    tokens_pm = np.ascontiguousarray(tokens.reshape(ST, P).T)
    base = dict(
        tokens_pm=tokens_pm,
        emb=emb16,
        posT=_dT_layout(pe * SX).astype(bf),
        # w1[fm, p, kd*128+e] = W1[kd*128+p, fm*128+e]
        w1=np.ascontiguousarray(
            W1.reshape(DT, P, FT, P).transpose(2, 1, 0, 3).reshape(FT, P, DT * P)
        ).astype(bf),
        b1=np.ascontiguousarray(np.asarray(inputs["b1"], np.float32).reshape(FT, P).T),
        cs=np.ascontiguousarray(W1.sum(axis=0).reshape(FT, P).T),
        # w2[kf, p, d] = W2[kf*128+p, d]
        w2=np.ascontiguousarray(W2.reshape(FT, P, D)).astype(bf),
        b2=np.ascontiguousarray(np.asarray(inputs["b2"], np.float32).reshape(1, D)),
    )

    Wq = np.asarray(inputs["Wq"], np.float32)
    Wk = np.asarray(inputs["Wk"], np.float32)
    Wv = np.asarray(inputs["Wv"], np.float32)
    bq = np.asarray(inputs["bq"], np.float32)
    bk = np.asarray(inputs["bk"], np.float32)
    bv = np.asarray(inputs["bv"], np.float32)

    def _wqk_layout(W):
        # [m, p, kd*128+e] = W[kd*128+p, m*128+e]
        return np.ascontiguousarray(
            W.reshape(DT, P, DT, P).transpose(2, 1, 0, 3).reshape(DT, P, DT * P)
        )

    def _wv_layout(W):
        # [n2, p, kd*512+e] = W[kd*128+p, n2*512+e]
        return np.ascontiguousarray(
            W.reshape(DT, P, 2, SB).transpose(2, 1, 0, 3).reshape(2, P, DT * SB)
        )

    # full x (fp32): for the generic q/k-bias cross term and the Wv mean-correction
    x_f = emb_f[tokens] + pe
    xbar = x_f.mean(axis=0)
    # summed v-bias (all heads) incl the fp8 mean-correction, folded into the residual
    bv_sum = np.zeros(D, np.float32)
    for c in range(NCORES):
        V8c = (Wv[c] * SV).astype(f8).astype(np.float32) / SV
        bv_sum += bv[c] + xbar @ (Wv[c] - V8c)
    bvs_arr = np.ascontiguousarray((bv_sum * SX).reshape(1, D))

    in_maps = []
    for c in range(NCORES):
        m = dict(base)
        rows = _owned_rows(c)
        tsl = tokens[rows]
        m["tokens_sl"] = np.ascontiguousarray(tsl.reshape(SLT, P).T)
        m["bvs"] = bvs_arr
        # pos rows for the residual, row layout [p, j*D+d] = pe[rows[128j+p], d]
        m["pos_slr"] = np.ascontiguousarray(
            (pe[rows, :] * SX).reshape(2, P, D).transpose(1, 0, 2).reshape(P, 2 * D)
        ).astype(bf)
        # M = Wq @ Wk^T, single fp8 at scale SW
        M = Wq[c] @ Wk[c].T
        m["m8"] = _wqk_layout((M * SW)).astype(f8)
        # Wv single fp8; mean quantization bias folded into bv
        V8 = (Wv[c] * SV).astype(f8)
        m["v8"] = _wv_layout(V8.astype(np.float32)).astype(f8)
        # exp bias: ln(8) + (x @ (Wk bq) + bq.bk)/32, arranged [p, t-tile]
        w_t = x_f @ (Wk[c] @ bq[c]) + float(bq[c] @ bk[c])
        wexp_v = np.log(SP8) + w_t / 32.0
        m["wexp"] = np.ascontiguousarray(wexp_v.reshape(ST, P).T).astype(np.float32)
        in_maps.append(m)
    return in_maps


def kernel(**inputs) -> np.ndarray:
    from concourse.bass_utils import run_bass_kernel_spmd

    nc = _get_module()
    in_maps = _prepare_in_maps(inputs)
    res = run_bass_kernel_spmd(nc, in_maps, core_ids=list(range(NCORES)))
    outp = np.empty((S, D), np.float32)
    for c in range(NCORES):
        outp[_owned_rows(c)] = res.results[c]["out"]
    return outp
